# revision 20
# baseline (speedup 1.0000x reference)
"""Tensor-parallel fused attention kernel for Trainium2 (8 NeuronCores).

Sharding: DP=2 over batch x TP=4 over kv-head pairs. Each core computes
q/k/v projections + RoPE + causal attention + output projection for its
(batch, 2 kv heads) shard in bf16, then a 4-core ReduceScatter combines
the partial output projections; the host assembles the disjoint row
shards into the full [2, 2048, 4096] output.

Attention uses transposed scores (sT = k^T q) so P^T is produced
directly by exp with no transpose matmuls; softmax denominators come
from an all-ones stationary matmul accumulated in PSUM, inverted via
exp(-ln(x)) on the scalar engine (same activation table set as exp).
Supertiles are processed largest-first so the final ReduceScatter chunk
is the cheapest one.
"""
import sys

for _p in ("/opt/trn_rl_repo", "/root/.axon_site/_ro/trn_rl_repo"):
    if _p not in sys.path:
        sys.path.append(_p)

import math
import numpy as np
import ml_dtypes

import concourse.bass as bass
import concourse.mybir as mybir
import concourse.tile as tile
from concourse import bacc
from concourse import bass_utils
from concourse.masks import make_identity

BF16 = ml_dtypes.bfloat16
FP32 = mybir.dt.float32
BF = mybir.dt.bfloat16

B, S, D = 2, 2048, 4096
R, K, H = 4, 8, 128
N_CORES = 8
TP = 4            # tensor-parallel ways (kv-head axis)
KLOC = K // TP    # kv heads per core = 2
HEADS = R * KLOC  # query heads per core = 8
DT = D // 128     # 32 d-tiles
ST = S // 128     # 16 s-tiles
NG = ST // 4      # 4 supertiles of 512 rows

# causal path: supertiles big-to-small; output chunks in execution order so
# the last ReduceScatter (the exposed tail) is a single row-tile.
G_ORDER = [3, 2, 1, 0]
# out-proj tiles are emitted interleaved with attention in supertile order
# (12..15, 8..11, 4..7, 0..3); 2-tile ReduceScatter chunks fire as their
# second tile completes so the collective queue starts as early as possible
# (it is the critical path: ~14.5us/MB + ~5us/chunk handoff). The last two
# chunks are single tiles to minimize the exposed tail.
CH_NEW = [(12, 2), (14, 2), (8, 2), (10, 2), (4, 2), (6, 2),
          (0, 1), (1, 1), (2, 1), (3, 1)]
CHUNK_OF_TILE = {}
for _ci, (_cs, _cn) in enumerate(CH_NEW):
    for _i in range(_cs, _cs + _cn):
        CHUNK_OF_TILE[_i] = _ci

# fallback (non-causal) chunks from the generic path
CC_CHUNKS = [(0, 2), (2, 2), (4, 2), (6, 2), (8, 2), (10, 2), (12, 2), (14, 1), (15, 1)]

_CACHE = {}


def _build_causal():
    nc = bacc.Bacc("TRN2", target_bir_lowering=False, debug=False,
                   enable_asserts=False, num_devices=N_CORES)

    xP = nc.dram_tensor("xP", [128, 2 * DT * (S // 2)], BF, kind="ExternalInput")
    wq = nc.dram_tensor("wq", [HEADS * 128, DT * 128], BF, kind="ExternalInput")
    wk = nc.dram_tensor("wk", [KLOC * 128, DT * 128], BF, kind="ExternalInput")
    wv = nc.dram_tensor("wv", [128, DT * KLOC * H], BF, kind="ExternalInput")
    wo = nc.dram_tensor("wo", [HEADS * H, D], BF, kind="ExternalInput")
    cosT = nc.dram_tensor("cosT", [H, S], FP32, kind="ExternalInput")
    sinST = nc.dram_tensor("sinST", [H, S], FP32, kind="ExternalInput")
    maskdT = nc.dram_tensor("maskdT", [128, S], FP32, kind="ExternalInput")
    out_sh = nc.dram_tensor("out_shard", [S // TP, D], BF, kind="ExternalOutput")

    with tile.TileContext(nc) as tc:
        with tc.tile_pool(name="persist", bufs=1) as persist, \
             tc.tile_pool(name="dram", bufs=1, space="DRAM") as dram:

            kT_t = [persist.tile([128, S], BF, tag=f"kT{i}", name=f"kT{i}")
                    for i in range(KLOC)]
            v_t = [persist.tile([128, KLOC * H], BF, tag=f"v{i}", name=f"v{i}")
                   for i in range(ST)]
            # qT in head-supertile-major layout: [H, (g, head, 512)]
            qg = persist.tile([128, NG * HEADS * 512], BF, tag="qg", name="qg")
            cc_in = [dram.tile([n * 128, D], BF, tag=f"ccin{g}", name=f"cc_in{g}")
                     for g, (st0, n) in enumerate(CH_NEW)]
            cc_out = [dram.tile([n * 32, D], BF, tag=f"ccout{g}", name=f"cc_out{g}")
                      for g, (st0, n) in enumerate(CH_NEW)]

            # ---------------- Phase 1: projections + rope ----------------
            with tc.tile_pool(name="p1", bufs=1) as p1, \
                 tc.tile_pool(name="p1ps", bufs=1, space="PSUM") as p1ps:
                ct = p1.tile([H, S], FP32, tag="ct")
                st_ = p1.tile([H, S], FP32, tag="st")
                wv_sb = p1.tile([128, DT * KLOC * H], BF, tag="wvsb")

                for half in range(2):
                    scol0 = half * (S // 2)
                    xth_t = [p1.tile([128, 8 * (S // 2)], BF, tag="xth", bufs=5,
                                     name=f"xth{half}_{qq}") for qq in range(4)]

                    def xth_dma(qq, split=False):
                        base = (half * DT + qq * 8) * (S // 2)
                        if split:
                            hw_ = 4 * (S // 2)
                            nc.sync.dma_start(xth_t[qq][:, :hw_],
                                              xP.ap()[:, base: base + hw_])
                            nc.sync.dma_start(xth_t[qq][:, hw_:],
                                              xP.ap()[:, base + hw_: base + 8 * (S // 2)])
                        else:
                            nc.sync.dma_start(
                                xth_t[qq][:],
                                xP.ap()[:, base: base + 8 * (S // 2)])

                    if half == 1:
                        for qq in range(4):
                            xth_dma(qq)

                    def xth(d, a, b):
                        return xth_t[d // 8][:, (d % 8) * (S // 2) + a:
                                             (d % 8) * (S // 2) + b]

                    # q (8 head-tiles) then k (KLOC head-tiles)
                    for h in range(HEADS + KLOC):
                        wsrc = wq.ap()[h * 128:(h + 1) * 128, :] if h < HEADS \
                            else wk.ap()[(h - HEADS) * 128:(h - HEADS + 1) * 128, :]
                        if half == 0 and h == 0:
                            xth_dma(0, split=True)
                        wslab = p1.tile([128, DT * 128], BF, tag="wslab", bufs=2)
                        nc.sync.dma_start(wslab[:], wsrc)
                        if half == 0 and h == 0:
                            # x quads go ahead of the rope tables / wv so the
                            # first projection chains aren't DMA-starved; the
                            # tables only gate the first rope (~17us in).
                            xth_dma(1)
                            nc.sync.dma_start(ct[:], cosT.ap())
                            nc.sync.dma_start(st_[:], sinST.ap())
                            xth_dma(2)
                            xth_dma(3)
                            nc.sync.dma_start(wv_sb[:], wv.ap())
                        for sc in range(2):  # 512-wide chunks within the half
                            lo = sc * 512
                            gcol = scol0 + lo
                            qp = p1ps.tile([128, 512], FP32, tag="qp", bufs=3)
                            for d in range(DT):
                                nc.tensor.matmul(
                                    qp[:],
                                    lhsT=wslab[:, d * 128:(d + 1) * 128],
                                    rhs=xth(d, lo, lo + 512),
                                    start=(d == 0), stop=(d == DT - 1))
                            # rope: out = qp*cos + rot(qp)*sin_signed
                            t1 = p1.tile([128, 512], FP32, tag="t1", bufs=2)
                            nc.vector.tensor_mul(t1[:], qp[:], ct[:, gcol:gcol + 512])
                            t2 = p1.tile([128, 512], FP32, tag="t2", bufs=2)
                            nc.vector.tensor_mul(t2[0:64, :], qp[64:128, :],
                                                 st_[0:64, gcol:gcol + 512])
                            nc.vector.tensor_mul(t2[64:128, :], qp[0:64, :],
                                                 st_[64:128, gcol:gcol + 512])
                            if h < HEADS:
                                g = gcol // 512
                                dst = (g * HEADS + h) * 512
                                nc.vector.tensor_add(qg[:, dst:dst + 512],
                                                     t1[:], t2[:])
                            else:
                                nc.vector.tensor_add(
                                    kT_t[h - HEADS][:, gcol:gcol + 512], t1[:], t2[:])

                    # v projection for the 8 s-tiles of this half
                    for stl in range(ST // 2):
                        sti = half * (ST // 2) + stl
                        vp = p1ps.tile([128, KLOC * H], FP32, tag="vp", bufs=2)
                        for d in range(DT):
                            nc.tensor.matmul(
                                vp[:],
                                lhsT=xth(d, stl * 128, (stl + 1) * 128),
                                rhs=wv_sb[:, d * KLOC * H:(d + 1) * KLOC * H],
                                start=(d == 0), stop=(d == DT - 1))
                        nc.scalar.copy(v_t[sti][:], vp[:])

            # ---------------- Phase 2: attention + out-proj ----------------
            with tc.tile_pool(name="p2", bufs=1) as p2, \
                 tc.tile_pool(name="p2ps", bufs=1, space="PSUM") as p2ps:
                ones = p2.tile([128, 128], BF, tag="ones")
                nc.gpsimd.memset(ones[:], 1.0)
                md = p2.tile([128, S], FP32, tag="maskdT")
                nc.sync.dma_start(md[:], maskdT.ap())
                wo_sb = [p2.tile([128, D], BF, tag=f"wo{i}", name=f"wo{i}")
                         for i in range(HEADS)]
                for i in range(HEADS):
                    nc.sync.dma_start(wo_sb[i][:], wo.ap()[i * 128:(i + 1) * 128, :])

                # yT for all 16 row-tiles, written per (supertile, head)
                yt_all = [p2.tile([128, S], BF, tag=f"yta{h}", name=f"yta{h}")
                          for h in range(HEADS)]

                chunk_emitted = [0] * len(CH_NEW)

                def oproj_tile(i):
                    """out-projection for row-tile i; fires the chunk's
                    ReduceScatter when its last tile completes."""
                    ci = CHUNK_OF_TILE[i]
                    cs, cn = CH_NEW[ci]
                    for dc in range(8):
                        op = p2ps.tile([128, 512], FP32, tag="op", bufs=2)
                        for head in range(HEADS):
                            nc.tensor.matmul(
                                op[:],
                                lhsT=yt_all[head][:, i * 128:(i + 1) * 128],
                                rhs=wo_sb[head][:, dc * 512:(dc + 1) * 512],
                                start=(head == 0), stop=(head == HEADS - 1))
                        # single-engine evac (mixing engines couples the oev
                        # buffer ring into a serializing cross-engine chain);
                        # DMA on the SP queue, which carries nothing else here.
                        oev = p2.tile([128, 512], BF, tag="oev", bufs=12)
                        nc.vector.tensor_copy(oev[:], op[:])
                        nc.sync.dma_start(
                            cc_in[ci][(i - cs) * 128:(i - cs + 1) * 128,
                                      dc * 512:(dc + 1) * 512],
                            oev[:])
                    chunk_emitted[ci] += 1
                    if chunk_emitted[ci] == cn:
                        nc.gpsimd.collective_compute(
                            "ReduceScatter", mybir.AluOpType.add,
                            replica_groups=[[0, 1, 2, 3], [4, 5, 6, 7]],
                            ins=[cc_in[ci].opt()], outs=[cc_out[ci].opt()])

                backlog = []

                # -------- attention, with out-proj tiles interleaved --------
                for g in G_ORDER:
                    jmax = 4 * g + 3
                    for head in range(HEADS):
                        kv = head % KLOC
                        qbase = (g * HEADS + head) * 512
                        rsum = p2ps.tile([128, 512], FP32, tag="rsum", bufs=1)
                        ypsum = p2ps.tile([128, 512], FP32, tag="ypsum", bufs=1)
                        # software pipeline: consume (ones/y) runs one pair
                        # behind produce (scores+exp) so the in-order PE never
                        # waits on the activation latency.
                        pending = None

                        def consume(ent):
                            pw_, los_, pbase = ent
                            for hh in range(2):
                                j = 2 * pbase + hh
                                lo = los_[hh]
                                nc.tensor.matmul(
                                    rsum[:, lo:512], lhsT=ones[:],
                                    rhs=pw_[:, hh * 512 + lo: hh * 512 + 512],
                                    start=(j == 0), stop=(j == jmax))
                                nc.tensor.matmul(
                                    ypsum[:, lo:512],
                                    lhsT=v_t[j][:, kv * H:(kv + 1) * H],
                                    rhs=pw_[:, hh * 512 + lo: hh * 512 + 512],
                                    start=(j == 0), stop=(j == jmax))

                        for p in range(2 * g + 2):
                            st2 = p2ps.tile([128, 1024], FP32, tag="st2", bufs=2)
                            pw = p2.tile([128, 1024], BF, tag="pw", bufs=3)
                            los = []
                            for hh in range(2):
                                j = 2 * p + hh
                                lo = max(0, j * 128 - g * 512)
                                los.append(lo)
                                nc.tensor.matmul(
                                    st2[:, hh * 512 + lo: hh * 512 + 512],
                                    lhsT=kT_t[kv][:, j * 128:(j + 1) * 128],
                                    rhs=qg[:, qbase + lo: qbase + 512],
                                    start=True, stop=True)
                                if j >= 4 * g:  # diagonal block: causal mask
                                    nc.vector.tensor_add(
                                        st2[:, hh * 512 + lo: hh * 512 + lo + 128],
                                        st2[:, hh * 512 + lo: hh * 512 + lo + 128],
                                        md[:, j * 128:(j + 1) * 128])
                            if los[0] == 0 and los[1] == 0:
                                nc.scalar.activation(
                                    pw[:], st2[:],
                                    mybir.ActivationFunctionType.Exp)
                            else:
                                for hh in range(2):
                                    lo = los[hh]
                                    nc.scalar.activation(
                                        pw[:, hh * 512 + lo: hh * 512 + 512],
                                        st2[:, hh * 512 + lo: hh * 512 + 512],
                                        mybir.ActivationFunctionType.Exp)
                            if pending is not None:
                                consume(pending)
                            pending = (pw, los, p)
                        consume(pending)
                        # softmax normalization: yT *= 1/rsum (fast NR approx)
                        rcb = p2.tile([128, 512], FP32, tag="rcb", bufs=2)
                        nc.vector.reciprocal_approx_fast(rcb[:], rsum[:])
                        nc.vector.tensor_mul(
                            yt_all[head][:, g * 512:(g + 1) * 512],
                            ypsum[:], rcb[:])
                        # interleave a ready out-proj tile after every other
                        # head so its ReduceScatter chunk enters the (critical
                        # path) collective queue as early as possible
                        if head % 2 == 1 and backlog:
                            oproj_tile(backlog.pop(0))
                    backlog += [4 * g, 4 * g + 1, 4 * g + 2, 4 * g + 3]

                # -------- remaining out-proj tiles --------
                while backlog:
                    oproj_tile(backlog.pop(0))
                # out_sh copies: on the GPSIMD (SWDGE) queue so the
                # collective->copy dependency is plain program order (a
                # SP-queue copy would need a completion-relay DMA that
                # head-of-line blocks the shared HWDGE semaphore ring), and
                # AFTER all collective triggers so a copy's in-queue wait
                # never delays the next ReduceScatter trigger.
                for ci, (cs, cn) in enumerate(CH_NEW):
                    orow = sum(cn2 * 32 for _, cn2 in CH_NEW[:ci])
                    nc.gpsimd.dma_start(
                        out_sh.ap()[orow: orow + cn * 32, :], cc_out[ci][:])

    nc.compile()
    return nc


def _build_generic():
    """Fallback for non-causal masks (never hit by the reference inputs):
    full-mask attention via the original row-major scores + PE-transpose
    scheme."""
    causal = False
    nc = bacc.Bacc("TRN2", target_bir_lowering=False, debug=False,
                   enable_asserts=False, num_devices=N_CORES)

    xP = nc.dram_tensor("xP", [128, 2 * DT * (S // 2)], BF, kind="ExternalInput")
    wq = nc.dram_tensor("wq", [HEADS * 128, DT * 128], BF, kind="ExternalInput")
    wk = nc.dram_tensor("wk", [KLOC * 128, DT * 128], BF, kind="ExternalInput")
    wv = nc.dram_tensor("wv", [128, DT * KLOC * H], BF, kind="ExternalInput")
    wo = nc.dram_tensor("wo", [HEADS * H, D], BF, kind="ExternalInput")
    cosT = nc.dram_tensor("cosT", [H, S], FP32, kind="ExternalInput")
    sinST = nc.dram_tensor("sinST", [H, S], FP32, kind="ExternalInput")
    maskf = nc.dram_tensor("maskf", [S, S], FP32, kind="ExternalInput")
    out_sh = nc.dram_tensor("out_shard", [S // TP, D], BF, kind="ExternalOutput")

    with tile.TileContext(nc) as tc:
        with tc.tile_pool(name="persist", bufs=1) as persist, \
             tc.tile_pool(name="dram", bufs=1, space="DRAM") as dram:

            kT_t = [persist.tile([128, S], BF, tag=f"kT{i}", name=f"kT{i}")
                    for i in range(KLOC)]
            v_t = [persist.tile([128, KLOC * H], BF, tag=f"v{i}", name=f"v{i}")
                   for i in range(ST)]
            wo_sb = [persist.tile([128, D], BF, tag=f"wo{i}", name=f"wo{i}")
                     for i in range(HEADS)]
            qT_dram = dram.tile([HEADS * 128, S], BF, tag="qtd", name="qT_dram")
            cc_in = [dram.tile([n * 128, D], BF, tag=f"ccin{g}", name=f"cc_in{g}")
                     for g, (st0, n) in enumerate(CC_CHUNKS)]
            cc_out = [dram.tile([n * 32, D], BF, tag=f"ccout{g}", name=f"cc_out{g}")
                      for g, (st0, n) in enumerate(CC_CHUNKS)]

            # ---------------- Phase 1: projections + rope ----------------
            with tc.tile_pool(name="p1", bufs=1) as p1, \
                 tc.tile_pool(name="p1ps", bufs=1, space="PSUM") as p1ps:
                ct = p1.tile([H, S], FP32, tag="ct")
                st = p1.tile([H, S], FP32, tag="st")
                wv_sb = p1.tile([128, DT * KLOC * H], BF, tag="wvsb")

                for half in range(2):
                    scols = (half * (S // 2), (half + 1) * (S // 2))
                    xth_t = [p1.tile([128, 8 * (S // 2)], BF, tag="xth", bufs=4,
                                     name=f"xth{half}_{qq}") for qq in range(4)]

                    def xth_dma(qq, split=False):
                        base = (half * DT + qq * 8) * (S // 2)
                        if split:
                            hw_ = 4 * (S // 2)
                            nc.sync.dma_start(xth_t[qq][:, :hw_],
                                              xP.ap()[:, base: base + hw_])
                            nc.sync.dma_start(xth_t[qq][:, hw_:],
                                              xP.ap()[:, base + hw_: base + 8 * (S // 2)])
                        else:
                            nc.sync.dma_start(
                                xth_t[qq][:],
                                xP.ap()[:, base: base + 8 * (S // 2)])

                    if half == 1:
                        for qq in range(4):
                            xth_dma(qq)

                    def xth(d, a, b):
                        return xth_t[d // 8][:, (d % 8) * (S // 2) + a:
                                             (d % 8) * (S // 2) + b]

                    for h in range(HEADS + KLOC):
                        wsrc = wq.ap()[h * 128:(h + 1) * 128, :] if h < HEADS \
                            else wk.ap()[(h - HEADS) * 128:(h - HEADS + 1) * 128, :]
                        if half == 0 and h == 0:
                            xth_dma(0, split=True)
                        wslab = p1.tile([128, DT * 128], BF, tag="wslab", bufs=2)
                        nc.sync.dma_start(wslab[:], wsrc)
                        if half == 0 and h == 0:
                            nc.sync.dma_start(ct[:], cosT.ap())
                            nc.sync.dma_start(st[:], sinST.ap())
                            for qq in range(1, 4):
                                xth_dma(qq)
                            nc.sync.dma_start(wv_sb[:], wv.ap())
                        for sc in range(2):
                            lo = sc * 512
                            qp = p1ps.tile([128, 512], FP32, tag="qp", bufs=3)
                            for d in range(DT):
                                nc.tensor.matmul(
                                    qp[:],
                                    lhsT=wslab[:, d * 128:(d + 1) * 128],
                                    rhs=xth(d, lo, lo + 512),
                                    start=(d == 0), stop=(d == DT - 1))
                            gcol = scols[0] + lo
                            t1 = p1.tile([128, 512], FP32, tag="t1", bufs=2)
                            nc.vector.tensor_mul(t1[:], qp[:], ct[:, gcol:gcol + 512])
                            t2 = p1.tile([128, 512], FP32, tag="t2", bufs=2)
                            nc.vector.tensor_mul(t2[0:64, :], qp[64:128, :],
                                                 st[0:64, gcol:gcol + 512])
                            nc.vector.tensor_mul(t2[64:128, :], qp[0:64, :],
                                                 st[64:128, gcol:gcol + 512])
                            if h < HEADS:
                                robf = p1.tile([128, 512], BF, tag="robf", bufs=2)
                                nc.vector.tensor_add(robf[:], t1[:], t2[:])
                                nc.sync.dma_start(
                                    qT_dram[h * 128:(h + 1) * 128, gcol:gcol + 512],
                                    robf[:])
                            else:
                                nc.vector.tensor_add(
                                    kT_t[h - HEADS][:, gcol:gcol + 512], t1[:], t2[:])

                    for stl in range(ST // 2):
                        sti = half * (ST // 2) + stl
                        vp = p1ps.tile([128, KLOC * H], FP32, tag="vp", bufs=2)
                        for d in range(DT):
                            nc.tensor.matmul(
                                vp[:],
                                lhsT=xth(d, stl * 128, (stl + 1) * 128),
                                rhs=wv_sb[:, d * KLOC * H:(d + 1) * KLOC * H],
                                start=(d == 0), stop=(d == DT - 1))
                        nc.scalar.copy(v_t[sti][:], vp[:])

                for i in range(HEADS):
                    nc.sync.dma_start(wo_sb[i][:], wo.ap()[i * 128:(i + 1) * 128, :])

            # ---------------- Phase 2: attention + out-proj ----------------
            with tc.tile_pool(name="p2", bufs=1) as p2, \
                 tc.tile_pool(name="p2ps", bufs=1, space="PSUM") as p2ps:
                ident = p2.tile([128, 128], BF, tag="ident")
                make_identity(nc, ident[:])

                qg_all = [p2.tile([128, S], BF, tag=f"qga{h}", name=f"qga{h}")
                          for h in range(HEADS)]
                for h in range(HEADS):
                    nc.sync.dma_start(qg_all[h][:], qT_dram[h * 128:(h + 1) * 128, :])

                segs = [(0, 4), (4, 4), (8, 4), (12, 4)]
                for t0, nt in segs:
                    W = nt * 128
                    mrow = [p2.tile([128, S], FP32, tag="mrow", bufs=4,
                                    name=f"mrow{t0}_{it}") for it in range(nt)]
                    for it in range(nt):
                        i = t0 + it
                        nc.sync.dma_start(mrow[it][:], maskf.ap()[i * 128:(i + 1) * 128, :])

                    yT_sb = [p2.tile([128, W], BF, tag=f"yt{h}", bufs=2,
                                     name=f"yt{t0}_{h}") for h in range(HEADS)]
                    for h in range(HEADS):
                        kv = h % KLOC
                        nquad = NG
                        pTq = [p2.tile([128, 4 * W], BF, tag=f"ptq{q}", bufs=1,
                                       name=f"ptq{t0}_{h}_{q}") for q in range(nquad)]
                        for it in range(nt):
                            i = t0 + it
                            nsk = S
                            prow = p2.tile([128, S], BF, tag="prow", bufs=3)
                            sums = []
                            nch = (nsk + 1023) // 1024
                            for c in range(nch):
                                w = min(1024, nsk - c * 1024)
                                sp = p2ps.tile([128, 1024], FP32, tag="sp", bufs=2)
                                for cc in range((w + 511) // 512):
                                    ww = min(512, w - cc * 512)
                                    o = cc * 512
                                    nc.tensor.matmul(
                                        sp[:, o:o + ww],
                                        lhsT=qg_all[h][:, i * 128:(i + 1) * 128],
                                        rhs=kT_t[kv][:, c * 1024 + o: c * 1024 + o + ww],
                                        start=True, stop=True)
                                nc.vector.tensor_add(
                                    sp[:, :w], sp[:, :w],
                                    mrow[it][:, c * 1024: c * 1024 + w])
                                sm = p2.tile([128, 1], FP32, tag="sm", bufs=8)
                                nc.scalar.activation(
                                    prow[:, c * 1024: c * 1024 + w], sp[:, :w],
                                    mybir.ActivationFunctionType.Exp, accum_out=sm[:])
                                sums.append(sm)
                            if nch == 2:
                                tot = p2.tile([128, 1], FP32, tag="tot", bufs=4)
                                nc.vector.tensor_add(tot[:], sums[0][:], sums[1][:])
                            else:
                                tot = sums[0]
                            rc = p2.tile([128, 1], FP32, tag="rc", bufs=4)
                            nc.vector.reciprocal(rc[:], tot[:])
                            diag = p2.tile([128, 128], BF, tag="diag", bufs=4)
                            nc.vector.tensor_scalar_mul(diag[:], ident[:], rc[:])
                            jtop = ST - 1
                            for q in range(jtop // 4 + 1):
                                jlo, jhi = 4 * q, min(4 * q + 3, jtop)
                                nq = jhi - jlo + 1
                                tpp = p2ps.tile([128, 512], FP32, tag="tp", bufs=2)
                                for j in range(jlo, jhi + 1):
                                    nc.tensor.matmul(
                                        tpp[:, (j - jlo) * 128:(j - jlo + 1) * 128],
                                        lhsT=prow[:, j * 128:(j + 1) * 128],
                                        rhs=diag[:], start=True, stop=True)
                                pt_dst = pTq[q][:].rearrange("p (a b) -> p a b", a=4)[
                                    :, 0:nq, it * 128:(it + 1) * 128]
                                pt_src = tpp[:, :nq * 128].rearrange(
                                    "p (a b) -> p a b", b=128)
                                if (it + q) % 2:
                                    nc.scalar.copy(pt_dst, pt_src)
                                else:
                                    nc.vector.tensor_copy(pt_dst, pt_src)
                        yp = p2ps.tile([128, W], FP32, tag="yp", bufs=1)
                        jmax = ST
                        for j in range(jmax):
                            lo = 0
                            nc.tensor.matmul(
                                yp[:, lo:W],
                                lhsT=v_t[j][:, kv * H:(kv + 1) * H],
                                rhs=pTq[j // 4][:, (j % 4) * W + lo: (j % 4) * W + W],
                                start=(j == 0), stop=(j == jmax - 1))
                        nc.scalar.copy(yT_sb[h][:], yp[:])

                    for it in range(nt):
                        i = t0 + it
                        cg = next(ci for ci, (cs, cn) in enumerate(CC_CHUNKS)
                                  if cs <= i < cs + cn)
                        c_start, c_n = CC_CHUNKS[cg]
                        for dc in range(8):
                            op = p2ps.tile([128, 512], FP32, tag="op", bufs=1)
                            for hh in range(HEADS):
                                nc.tensor.matmul(
                                    op[:],
                                    lhsT=yT_sb[hh][:, it * 128:(it + 1) * 128],
                                    rhs=wo_sb[hh][:, dc * 512:(dc + 1) * 512],
                                    start=(hh == 0), stop=(hh == HEADS - 1))
                            oev = p2.tile([128, 512], BF, tag="oev", bufs=16)
                            nc.scalar.copy(oev[:], op[:])
                            nc.sync.dma_start(
                                cc_in[cg][(i - c_start) * 128:(i - c_start + 1) * 128,
                                          dc * 512:(dc + 1) * 512],
                                oev[:])
                        if i == c_start + c_n - 1:
                            nc.gpsimd.collective_compute(
                                "ReduceScatter", mybir.AluOpType.add,
                                replica_groups=[[0, 1, 2, 3], [4, 5, 6, 7]],
                                ins=[cc_in[cg].opt()], outs=[cc_out[cg].opt()])
                            orow = sum(cn * 32 for cs, cn in CC_CHUNKS[:cg])
                            nc.sync.dma_start(
                                out_sh.ap()[orow: orow + c_n * 32, :], cc_out[cg][:])

    nc.compile()
    return nc


_CANON_MASK = None


def _is_causal(mask: np.ndarray) -> bool:
    global _CANON_MASK
    if _CANON_MASK is None:
        _CANON_MASK = np.triu(np.full((S, S), -1e9, dtype=np.float32), k=1)
    return mask.shape == (S, S) and np.array_equal(mask, _CANON_MASK)


def _prepare(x, wq, wk, wv, wo, mask, sin, cos):
    causal = _is_causal(np.asarray(mask, dtype=np.float32))
    if causal not in _CACHE:
        _CACHE[causal] = _build_causal() if causal else _build_generic()
    nc = _CACHE[causal]
    chunks = CH_NEW if causal else CC_CHUNKS

    x = np.asarray(x, dtype=np.float32)
    scale = np.float32(H ** -0.5)
    cosT = np.ascontiguousarray(np.asarray(cos, np.float32).T)          # [H, S]
    sinT = np.asarray(sin, np.float32).T.copy()                          # [H, S]
    sinT[0:H // 2] = -sinT[0:H // 2]                                     # signed
    # per-core weight shards; head order = r-major over local kv heads
    in_maps = []
    for c in range(N_CORES):
        b, tp = c // TP, c % TP
        ks = slice(tp * KLOC, (tp + 1) * KLOC)
        wq_c = np.asarray(wq, np.float32)[:, :, ks, :].reshape(D, HEADS * H)
        wk_c = (np.asarray(wk, np.float32)[:, ks, :] * scale).reshape(D, KLOC * H)
        wv_c = np.asarray(wv, np.float32)[:, ks, :].reshape(D, KLOC * H)
        m = {
            "xP": x[b].reshape(2, S // 2, DT, 128).transpose(3, 0, 2, 1)
                     .reshape(128, 2 * DT * (S // 2)).astype(BF16),
            "wq": wq_c.reshape(DT, 128, HEADS, H).transpose(2, 1, 0, 3)
                      .reshape(HEADS * 128, DT * 128).astype(BF16),
            "wk": wk_c.reshape(DT, 128, KLOC, H).transpose(2, 1, 0, 3)
                      .reshape(KLOC * 128, DT * 128).astype(BF16),
            "wv": wv_c.reshape(DT, 128, KLOC * H).transpose(1, 0, 2)
                      .reshape(128, DT * KLOC * H).astype(BF16),
            "wo": np.asarray(wo, np.float32)[:, ks, :, :].reshape(HEADS * H, D).astype(BF16),
            "cosT": cosT,
            "sinST": sinT,
        }
        if causal:
            mdT = np.empty((128, S), np.float32)
            for i in range(ST):
                blk = mask[i * 128:(i + 1) * 128, i * 128:(i + 1) * 128]
                mdT[:, i * 128:(i + 1) * 128] = np.ascontiguousarray(blk.T)
            m["maskdT"] = mdT
        else:
            m["maskf"] = np.asarray(mask, np.float32)
        in_maps.append(m)
    return nc, in_maps, chunks


def _assemble(results, chunks):
    out = np.empty((B, S, D), dtype=np.float32)
    for c in range(N_CORES):
        b, tp = c // TP, c % TP
        sh = results[c]["out_shard"].astype(np.float32)
        orow = 0
        for cs, cn in chunks:
            rows = cn * 32
            out[b, cs * 128 + tp * rows: cs * 128 + (tp + 1) * rows, :] = \
                sh[orow: orow + rows]
            orow += rows
    return out


def kernel(x, wq, wk, wv, wo, mask, sin, cos):
    nc, in_maps, chunks = _prepare(x, wq, wk, wv, wo, mask, sin, cos)
    try:
        res = bass_utils.run_bass_kernel_spmd(nc, in_maps,
                                              core_ids=list(range(N_CORES)))
    except Exception:
        # transient device-side failures (e.g. NRT exec-unit errors) have
        # been observed once; a clean re-run succeeds.
        import time as _time
        _time.sleep(2.0)
        res = bass_utils.run_bass_kernel_spmd(nc, in_maps,
                                              core_ids=list(range(N_CORES)))
    return _assemble(res.results, chunks)


def _traced_run(x, wq, wk, wv, wo, mask, sin, cos):
    """Like kernel() but with NTFF tracing; returns BassKernelResults."""
    nc, in_maps, chunks = _prepare(x, wq, wk, wv, wo, mask, sin, cos)
    res = bass_utils.run_bass_kernel_spmd(nc, in_maps, core_ids=list(range(N_CORES)),
                                          trace=True)
    res.full_output = _assemble(res.results, chunks)
    return res


# revision 22
# speedup vs baseline: 1.0117x; 1.0117x over previous
"""Tensor-parallel fused attention kernel for Trainium2 (8 NeuronCores).

Sharding: DP=2 over batch x TP=4 over kv-head pairs. Each core computes
q/k/v projections + RoPE + causal attention + output projection for its
(batch, 2 kv heads) shard in bf16, then a 4-core ReduceScatter combines
the partial output projections; the host assembles the disjoint row
shards into the full [2, 2048, 4096] output.

Attention uses transposed scores (sT = k^T q) so P^T is produced
directly by exp with no transpose matmuls; softmax denominators come
from an all-ones stationary matmul accumulated in PSUM, inverted via
exp(-ln(x)) on the scalar engine (same activation table set as exp).
Supertiles are processed largest-first so the final ReduceScatter chunk
is the cheapest one.
"""
import sys

for _p in ("/opt/trn_rl_repo", "/root/.axon_site/_ro/trn_rl_repo"):
    if _p not in sys.path:
        sys.path.append(_p)

import math
import numpy as np
import ml_dtypes

import concourse.bass as bass
import concourse.mybir as mybir
import concourse.tile as tile
from concourse import bacc
from concourse import bass_utils
from concourse.masks import make_identity

BF16 = ml_dtypes.bfloat16
FP32 = mybir.dt.float32
BF = mybir.dt.bfloat16

B, S, D = 2, 2048, 4096
R, K, H = 4, 8, 128
N_CORES = 8
TP = 4            # tensor-parallel ways (kv-head axis)
KLOC = K // TP    # kv heads per core = 2
HEADS = R * KLOC  # query heads per core = 8
DT = D // 128     # 32 d-tiles
ST = S // 128     # 16 s-tiles
NG = ST // 4      # 4 supertiles of 512 rows

# causal path: supertiles big-to-small; output chunks in execution order so
# the last ReduceScatter (the exposed tail) is a single row-tile.
G_ORDER = [3, 2, 1, 0]
# out-proj tiles are emitted interleaved with attention in supertile order
# (12..15, 8..11, 4..7, 0..3); 2-tile ReduceScatter chunks fire as their
# second tile completes so the collective queue starts as early as possible
# (it is the critical path: ~14.5us/MB + ~5us/chunk handoff). The last two
# chunks are single tiles to minimize the exposed tail.
CH_NEW = [(12, 2), (14, 2), (8, 2), (10, 2), (4, 2), (6, 2),
          (0, 1), (1, 1), (2, 1), (3, 1)]
CHUNK_OF_TILE = {}
for _ci, (_cs, _cn) in enumerate(CH_NEW):
    for _i in range(_cs, _cs + _cn):
        CHUNK_OF_TILE[_i] = _ci

# fallback (non-causal) chunks from the generic path
CC_CHUNKS = [(0, 2), (2, 2), (4, 2), (6, 2), (8, 2), (10, 2), (12, 2), (14, 1), (15, 1)]

_CACHE = {}


def _build_causal():
    nc = bacc.Bacc("TRN2", target_bir_lowering=False, debug=False,
                   enable_asserts=False, num_devices=N_CORES)

    xP = nc.dram_tensor("xP", [128, 2 * DT * (S // 2)], BF, kind="ExternalInput")
    wq = nc.dram_tensor("wq", [HEADS * 128, DT * 128], BF, kind="ExternalInput")
    wk = nc.dram_tensor("wk", [KLOC * 128, DT * 128], BF, kind="ExternalInput")
    wv = nc.dram_tensor("wv", [128, DT * KLOC * H], BF, kind="ExternalInput")
    wo = nc.dram_tensor("wo", [HEADS * H, D], BF, kind="ExternalInput")
    cosT = nc.dram_tensor("cosT", [H, S], FP32, kind="ExternalInput")
    sinST = nc.dram_tensor("sinST", [H, S], FP32, kind="ExternalInput")
    maskdT = nc.dram_tensor("maskdT", [128, S], FP32, kind="ExternalInput")
    out_sh = nc.dram_tensor("out_shard", [S // TP, D], BF, kind="ExternalOutput")

    with tile.TileContext(nc) as tc:
        with tc.tile_pool(name="persist", bufs=1) as persist, \
             tc.tile_pool(name="dram", bufs=1, space="DRAM") as dram:

            kT_t = [persist.tile([128, S], BF, tag=f"kT{i}", name=f"kT{i}")
                    for i in range(KLOC)]
            v_t = [persist.tile([128, KLOC * H], BF, tag=f"v{i}", name=f"v{i}")
                   for i in range(ST)]
            # qT in head-supertile-major layout: [H, (g, head, 512)]
            qg = persist.tile([128, NG * HEADS * 512], BF, tag="qg", name="qg")
            cc_in = [dram.tile([n * 128, D], BF, tag=f"ccin{g}", name=f"cc_in{g}")
                     for g, (st0, n) in enumerate(CH_NEW)]
            cc_out = [dram.tile([n * 32, D], BF, tag=f"ccout{g}", name=f"cc_out{g}")
                      for g, (st0, n) in enumerate(CH_NEW)]

            # ---------------- Phase 1: projections + rope ----------------
            with tc.tile_pool(name="p1", bufs=1) as p1, \
                 tc.tile_pool(name="p1ps", bufs=1, space="PSUM") as p1ps:
                ct = p1.tile([H, S], FP32, tag="ct")
                st_ = p1.tile([H, S], FP32, tag="st")
                wv_sb = p1.tile([128, DT * KLOC * H], BF, tag="wvsb")

                for half in range(2):
                    scol0 = half * (S // 2)
                    xth_t = [p1.tile([128, 8 * (S // 2)], BF, tag="xth", bufs=5,
                                     name=f"xth{half}_{qq}") for qq in range(4)]

                    def xth_dma(qq, split=False):
                        base = (half * DT + qq * 8) * (S // 2)
                        if split:
                            hw_ = 4 * (S // 2)
                            nc.sync.dma_start(xth_t[qq][:, :hw_],
                                              xP.ap()[:, base: base + hw_])
                            nc.sync.dma_start(xth_t[qq][:, hw_:],
                                              xP.ap()[:, base + hw_: base + 8 * (S // 2)])
                        else:
                            nc.sync.dma_start(
                                xth_t[qq][:],
                                xP.ap()[:, base: base + 8 * (S // 2)])

                    if half == 1:
                        for qq in range(4):
                            xth_dma(qq)

                    def xth(d, a, b):
                        return xth_t[d // 8][:, (d % 8) * (S // 2) + a:
                                             (d % 8) * (S // 2) + b]

                    # q (8 head-tiles) then k (KLOC head-tiles)
                    for h in range(HEADS + KLOC):
                        wsrc = wq.ap()[h * 128:(h + 1) * 128, :] if h < HEADS \
                            else wk.ap()[(h - HEADS) * 128:(h - HEADS + 1) * 128, :]
                        if half == 0 and h == 0:
                            xth_dma(0, split=True)
                        wslab = p1.tile([128, DT * 128], BF, tag="wslab", bufs=2)
                        nc.sync.dma_start(wslab[:], wsrc)
                        if half == 0 and h == 0:
                            nc.sync.dma_start(ct[:], cosT.ap())
                            nc.sync.dma_start(st_[:], sinST.ap())
                            for qq in range(1, 4):
                                xth_dma(qq)
                            nc.sync.dma_start(wv_sb[:], wv.ap())
                        for sc in range(2):  # 512-wide chunks within the half
                            lo = sc * 512
                            gcol = scol0 + lo
                            qp = p1ps.tile([128, 512], FP32, tag="qp", bufs=3)
                            for d in range(DT):
                                nc.tensor.matmul(
                                    qp[:],
                                    lhsT=wslab[:, d * 128:(d + 1) * 128],
                                    rhs=xth(d, lo, lo + 512),
                                    start=(d == 0), stop=(d == DT - 1))
                            # rope: out = qp*cos + rot(qp)*sin_signed
                            t1 = p1.tile([128, 512], FP32, tag="t1", bufs=2)
                            nc.vector.tensor_mul(t1[:], qp[:], ct[:, gcol:gcol + 512])
                            t2 = p1.tile([128, 512], FP32, tag="t2", bufs=2)
                            nc.vector.tensor_mul(t2[0:64, :], qp[64:128, :],
                                                 st_[0:64, gcol:gcol + 512])
                            nc.vector.tensor_mul(t2[64:128, :], qp[0:64, :],
                                                 st_[64:128, gcol:gcol + 512])
                            if h < HEADS:
                                g = gcol // 512
                                dst = (g * HEADS + h) * 512
                                nc.vector.tensor_add(qg[:, dst:dst + 512],
                                                     t1[:], t2[:])
                            else:
                                nc.vector.tensor_add(
                                    kT_t[h - HEADS][:, gcol:gcol + 512], t1[:], t2[:])

                    # v projection for the 8 s-tiles of this half
                    for stl in range(ST // 2):
                        sti = half * (ST // 2) + stl
                        vp = p1ps.tile([128, KLOC * H], FP32, tag="vp", bufs=2)
                        for d in range(DT):
                            nc.tensor.matmul(
                                vp[:],
                                lhsT=xth(d, stl * 128, (stl + 1) * 128),
                                rhs=wv_sb[:, d * KLOC * H:(d + 1) * KLOC * H],
                                start=(d == 0), stop=(d == DT - 1))
                        nc.scalar.copy(v_t[sti][:], vp[:])

            # ---------------- Phase 2: attention + out-proj ----------------
            with tc.tile_pool(name="p2", bufs=1) as p2, \
                 tc.tile_pool(name="p2ps", bufs=1, space="PSUM") as p2ps:
                ones = p2.tile([128, 128], BF, tag="ones")
                nc.gpsimd.memset(ones[:], 1.0)
                md = p2.tile([128, S], FP32, tag="maskdT")
                nc.sync.dma_start(md[:], maskdT.ap())
                wo_sb = [p2.tile([128, D], BF, tag=f"wo{i}", name=f"wo{i}")
                         for i in range(HEADS)]
                for i in range(HEADS):
                    nc.sync.dma_start(wo_sb[i][:], wo.ap()[i * 128:(i + 1) * 128, :])

                # yT for all 16 row-tiles, written per (supertile, head)
                yt_all = [p2.tile([128, S], BF, tag=f"yta{h}", name=f"yta{h}")
                          for h in range(HEADS)]

                chunk_emitted = [0] * len(CH_NEW)

                def oproj_tile(i):
                    """out-projection for row-tile i; fires the chunk's
                    ReduceScatter when its last tile completes."""
                    ci = CHUNK_OF_TILE[i]
                    cs, cn = CH_NEW[ci]
                    for dc in range(8):
                        op = p2ps.tile([128, 512], FP32, tag="op", bufs=2)
                        for head in range(HEADS):
                            nc.tensor.matmul(
                                op[:],
                                lhsT=yt_all[head][:, i * 128:(i + 1) * 128],
                                rhs=wo_sb[head][:, dc * 512:(dc + 1) * 512],
                                start=(head == 0), stop=(head == HEADS - 1))
                        # single-engine evac (mixing engines couples the oev
                        # buffer ring into a serializing cross-engine chain);
                        # DMA on the SP queue, which carries nothing else here.
                        oev = p2.tile([128, 512], BF, tag="oev", bufs=12)
                        nc.vector.tensor_copy(oev[:], op[:])
                        nc.sync.dma_start(
                            cc_in[ci][(i - cs) * 128:(i - cs + 1) * 128,
                                      dc * 512:(dc + 1) * 512],
                            oev[:])
                    chunk_emitted[ci] += 1
                    if chunk_emitted[ci] == cn:
                        nc.gpsimd.collective_compute(
                            "ReduceScatter", mybir.AluOpType.add,
                            replica_groups=[[0, 1, 2, 3], [4, 5, 6, 7]],
                            ins=[cc_in[ci].opt()], outs=[cc_out[ci].opt()])

                backlog = []

                # -------- attention, with out-proj tiles interleaved --------
                for g in G_ORDER:
                    jmax = 4 * g + 3
                    for head in range(HEADS):
                        kv = head % KLOC
                        qbase = (g * HEADS + head) * 512
                        rsum = p2ps.tile([128, 512], FP32, tag="rsum", bufs=1)
                        ypsum = p2ps.tile([128, 512], FP32, tag="ypsum", bufs=1)
                        # software pipeline: consume (ones/y) runs one pair
                        # behind produce (scores+exp) so the in-order PE never
                        # waits on the activation latency.
                        pending = None

                        def consume(ent):
                            pw_, los_, pbase = ent
                            for hh in range(2):
                                j = 2 * pbase + hh
                                lo = los_[hh]
                                nc.tensor.matmul(
                                    rsum[:, lo:512], lhsT=ones[:],
                                    rhs=pw_[:, hh * 512 + lo: hh * 512 + 512],
                                    start=(j == 0), stop=(j == jmax))
                                nc.tensor.matmul(
                                    ypsum[:, lo:512],
                                    lhsT=v_t[j][:, kv * H:(kv + 1) * H],
                                    rhs=pw_[:, hh * 512 + lo: hh * 512 + 512],
                                    start=(j == 0), stop=(j == jmax))

                        for p in range(2 * g + 2):
                            st2 = p2ps.tile([128, 1024], FP32, tag="st2", bufs=2)
                            pw = p2.tile([128, 1024], BF, tag="pw", bufs=3)
                            los = []
                            for hh in range(2):
                                j = 2 * p + hh
                                lo = max(0, j * 128 - g * 512)
                                los.append(lo)
                                nc.tensor.matmul(
                                    st2[:, hh * 512 + lo: hh * 512 + 512],
                                    lhsT=kT_t[kv][:, j * 128:(j + 1) * 128],
                                    rhs=qg[:, qbase + lo: qbase + 512],
                                    start=True, stop=True)
                                if j >= 4 * g:  # diagonal block: causal mask
                                    nc.vector.tensor_add(
                                        st2[:, hh * 512 + lo: hh * 512 + lo + 128],
                                        st2[:, hh * 512 + lo: hh * 512 + lo + 128],
                                        md[:, j * 128:(j + 1) * 128])
                            if los[0] == 0 and los[1] == 0:
                                nc.scalar.activation(
                                    pw[:], st2[:],
                                    mybir.ActivationFunctionType.Exp)
                            else:
                                for hh in range(2):
                                    lo = los[hh]
                                    nc.scalar.activation(
                                        pw[:, hh * 512 + lo: hh * 512 + 512],
                                        st2[:, hh * 512 + lo: hh * 512 + 512],
                                        mybir.ActivationFunctionType.Exp)
                            if pending is not None:
                                consume(pending)
                            pending = (pw, los, p)
                        consume(pending)
                        # softmax normalization: yT *= 1/rsum (fast NR approx)
                        rcb = p2.tile([128, 512], FP32, tag="rcb", bufs=2)
                        nc.vector.reciprocal_approx_fast(rcb[:], rsum[:])
                        nc.vector.tensor_mul(
                            yt_all[head][:, g * 512:(g + 1) * 512],
                            ypsum[:], rcb[:])
                        # interleave a ready out-proj tile after every head so
                        # its ReduceScatter chunk enters the (critical path)
                        # collective queue as early as possible
                        if backlog:
                            oproj_tile(backlog.pop(0))
                    backlog += [4 * g, 4 * g + 1, 4 * g + 2, 4 * g + 3]

                # -------- remaining out-proj tiles --------
                while backlog:
                    oproj_tile(backlog.pop(0))
                # out_sh copies: on the GPSIMD (SWDGE) queue so the
                # collective->copy dependency is plain program order (a
                # SP-queue copy would need a completion-relay DMA that
                # head-of-line blocks the shared HWDGE semaphore ring), and
                # AFTER all collective triggers so a copy's in-queue wait
                # never delays the next ReduceScatter trigger.
                for ci, (cs, cn) in enumerate(CH_NEW):
                    orow = sum(cn2 * 32 for _, cn2 in CH_NEW[:ci])
                    nc.gpsimd.dma_start(
                        out_sh.ap()[orow: orow + cn * 32, :], cc_out[ci][:])

    nc.compile()
    return nc


def _build_generic():
    """Fallback for non-causal masks (never hit by the reference inputs):
    full-mask attention via the original row-major scores + PE-transpose
    scheme."""
    causal = False
    nc = bacc.Bacc("TRN2", target_bir_lowering=False, debug=False,
                   enable_asserts=False, num_devices=N_CORES)

    xP = nc.dram_tensor("xP", [128, 2 * DT * (S // 2)], BF, kind="ExternalInput")
    wq = nc.dram_tensor("wq", [HEADS * 128, DT * 128], BF, kind="ExternalInput")
    wk = nc.dram_tensor("wk", [KLOC * 128, DT * 128], BF, kind="ExternalInput")
    wv = nc.dram_tensor("wv", [128, DT * KLOC * H], BF, kind="ExternalInput")
    wo = nc.dram_tensor("wo", [HEADS * H, D], BF, kind="ExternalInput")
    cosT = nc.dram_tensor("cosT", [H, S], FP32, kind="ExternalInput")
    sinST = nc.dram_tensor("sinST", [H, S], FP32, kind="ExternalInput")
    maskf = nc.dram_tensor("maskf", [S, S], FP32, kind="ExternalInput")
    out_sh = nc.dram_tensor("out_shard", [S // TP, D], BF, kind="ExternalOutput")

    with tile.TileContext(nc) as tc:
        with tc.tile_pool(name="persist", bufs=1) as persist, \
             tc.tile_pool(name="dram", bufs=1, space="DRAM") as dram:

            kT_t = [persist.tile([128, S], BF, tag=f"kT{i}", name=f"kT{i}")
                    for i in range(KLOC)]
            v_t = [persist.tile([128, KLOC * H], BF, tag=f"v{i}", name=f"v{i}")
                   for i in range(ST)]
            wo_sb = [persist.tile([128, D], BF, tag=f"wo{i}", name=f"wo{i}")
                     for i in range(HEADS)]
            qT_dram = dram.tile([HEADS * 128, S], BF, tag="qtd", name="qT_dram")
            cc_in = [dram.tile([n * 128, D], BF, tag=f"ccin{g}", name=f"cc_in{g}")
                     for g, (st0, n) in enumerate(CC_CHUNKS)]
            cc_out = [dram.tile([n * 32, D], BF, tag=f"ccout{g}", name=f"cc_out{g}")
                      for g, (st0, n) in enumerate(CC_CHUNKS)]

            # ---------------- Phase 1: projections + rope ----------------
            with tc.tile_pool(name="p1", bufs=1) as p1, \
                 tc.tile_pool(name="p1ps", bufs=1, space="PSUM") as p1ps:
                ct = p1.tile([H, S], FP32, tag="ct")
                st = p1.tile([H, S], FP32, tag="st")
                wv_sb = p1.tile([128, DT * KLOC * H], BF, tag="wvsb")

                for half in range(2):
                    scols = (half * (S // 2), (half + 1) * (S // 2))
                    xth_t = [p1.tile([128, 8 * (S // 2)], BF, tag="xth", bufs=4,
                                     name=f"xth{half}_{qq}") for qq in range(4)]

                    def xth_dma(qq, split=False):
                        base = (half * DT + qq * 8) * (S // 2)
                        if split:
                            hw_ = 4 * (S // 2)
                            nc.sync.dma_start(xth_t[qq][:, :hw_],
                                              xP.ap()[:, base: base + hw_])
                            nc.sync.dma_start(xth_t[qq][:, hw_:],
                                              xP.ap()[:, base + hw_: base + 8 * (S // 2)])
                        else:
                            nc.sync.dma_start(
                                xth_t[qq][:],
                                xP.ap()[:, base: base + 8 * (S // 2)])

                    if half == 1:
                        for qq in range(4):
                            xth_dma(qq)

                    def xth(d, a, b):
                        return xth_t[d // 8][:, (d % 8) * (S // 2) + a:
                                             (d % 8) * (S // 2) + b]

                    for h in range(HEADS + KLOC):
                        wsrc = wq.ap()[h * 128:(h + 1) * 128, :] if h < HEADS \
                            else wk.ap()[(h - HEADS) * 128:(h - HEADS + 1) * 128, :]
                        if half == 0 and h == 0:
                            xth_dma(0, split=True)
                        wslab = p1.tile([128, DT * 128], BF, tag="wslab", bufs=2)
                        nc.sync.dma_start(wslab[:], wsrc)
                        if half == 0 and h == 0:
                            nc.sync.dma_start(ct[:], cosT.ap())
                            nc.sync.dma_start(st[:], sinST.ap())
                            for qq in range(1, 4):
                                xth_dma(qq)
                            nc.sync.dma_start(wv_sb[:], wv.ap())
                        for sc in range(2):
                            lo = sc * 512
                            qp = p1ps.tile([128, 512], FP32, tag="qp", bufs=3)
                            for d in range(DT):
                                nc.tensor.matmul(
                                    qp[:],
                                    lhsT=wslab[:, d * 128:(d + 1) * 128],
                                    rhs=xth(d, lo, lo + 512),
                                    start=(d == 0), stop=(d == DT - 1))
                            gcol = scols[0] + lo
                            t1 = p1.tile([128, 512], FP32, tag="t1", bufs=2)
                            nc.vector.tensor_mul(t1[:], qp[:], ct[:, gcol:gcol + 512])
                            t2 = p1.tile([128, 512], FP32, tag="t2", bufs=2)
                            nc.vector.tensor_mul(t2[0:64, :], qp[64:128, :],
                                                 st[0:64, gcol:gcol + 512])
                            nc.vector.tensor_mul(t2[64:128, :], qp[0:64, :],
                                                 st[64:128, gcol:gcol + 512])
                            if h < HEADS:
                                robf = p1.tile([128, 512], BF, tag="robf", bufs=2)
                                nc.vector.tensor_add(robf[:], t1[:], t2[:])
                                nc.sync.dma_start(
                                    qT_dram[h * 128:(h + 1) * 128, gcol:gcol + 512],
                                    robf[:])
                            else:
                                nc.vector.tensor_add(
                                    kT_t[h - HEADS][:, gcol:gcol + 512], t1[:], t2[:])

                    for stl in range(ST // 2):
                        sti = half * (ST // 2) + stl
                        vp = p1ps.tile([128, KLOC * H], FP32, tag="vp", bufs=2)
                        for d in range(DT):
                            nc.tensor.matmul(
                                vp[:],
                                lhsT=xth(d, stl * 128, (stl + 1) * 128),
                                rhs=wv_sb[:, d * KLOC * H:(d + 1) * KLOC * H],
                                start=(d == 0), stop=(d == DT - 1))
                        nc.scalar.copy(v_t[sti][:], vp[:])

                for i in range(HEADS):
                    nc.sync.dma_start(wo_sb[i][:], wo.ap()[i * 128:(i + 1) * 128, :])

            # ---------------- Phase 2: attention + out-proj ----------------
            with tc.tile_pool(name="p2", bufs=1) as p2, \
                 tc.tile_pool(name="p2ps", bufs=1, space="PSUM") as p2ps:
                ident = p2.tile([128, 128], BF, tag="ident")
                make_identity(nc, ident[:])

                qg_all = [p2.tile([128, S], BF, tag=f"qga{h}", name=f"qga{h}")
                          for h in range(HEADS)]
                for h in range(HEADS):
                    nc.sync.dma_start(qg_all[h][:], qT_dram[h * 128:(h + 1) * 128, :])

                segs = [(0, 4), (4, 4), (8, 4), (12, 4)]
                for t0, nt in segs:
                    W = nt * 128
                    mrow = [p2.tile([128, S], FP32, tag="mrow", bufs=4,
                                    name=f"mrow{t0}_{it}") for it in range(nt)]
                    for it in range(nt):
                        i = t0 + it
                        nc.sync.dma_start(mrow[it][:], maskf.ap()[i * 128:(i + 1) * 128, :])

                    yT_sb = [p2.tile([128, W], BF, tag=f"yt{h}", bufs=2,
                                     name=f"yt{t0}_{h}") for h in range(HEADS)]
                    for h in range(HEADS):
                        kv = h % KLOC
                        nquad = NG
                        pTq = [p2.tile([128, 4 * W], BF, tag=f"ptq{q}", bufs=1,
                                       name=f"ptq{t0}_{h}_{q}") for q in range(nquad)]
                        for it in range(nt):
                            i = t0 + it
                            nsk = S
                            prow = p2.tile([128, S], BF, tag="prow", bufs=3)
                            sums = []
                            nch = (nsk + 1023) // 1024
                            for c in range(nch):
                                w = min(1024, nsk - c * 1024)
                                sp = p2ps.tile([128, 1024], FP32, tag="sp", bufs=2)
                                for cc in range((w + 511) // 512):
                                    ww = min(512, w - cc * 512)
                                    o = cc * 512
                                    nc.tensor.matmul(
                                        sp[:, o:o + ww],
                                        lhsT=qg_all[h][:, i * 128:(i + 1) * 128],
                                        rhs=kT_t[kv][:, c * 1024 + o: c * 1024 + o + ww],
                                        start=True, stop=True)
                                nc.vector.tensor_add(
                                    sp[:, :w], sp[:, :w],
                                    mrow[it][:, c * 1024: c * 1024 + w])
                                sm = p2.tile([128, 1], FP32, tag="sm", bufs=8)
                                nc.scalar.activation(
                                    prow[:, c * 1024: c * 1024 + w], sp[:, :w],
                                    mybir.ActivationFunctionType.Exp, accum_out=sm[:])
                                sums.append(sm)
                            if nch == 2:
                                tot = p2.tile([128, 1], FP32, tag="tot", bufs=4)
                                nc.vector.tensor_add(tot[:], sums[0][:], sums[1][:])
                            else:
                                tot = sums[0]
                            rc = p2.tile([128, 1], FP32, tag="rc", bufs=4)
                            nc.vector.reciprocal(rc[:], tot[:])
                            diag = p2.tile([128, 128], BF, tag="diag", bufs=4)
                            nc.vector.tensor_scalar_mul(diag[:], ident[:], rc[:])
                            jtop = ST - 1
                            for q in range(jtop // 4 + 1):
                                jlo, jhi = 4 * q, min(4 * q + 3, jtop)
                                nq = jhi - jlo + 1
                                tpp = p2ps.tile([128, 512], FP32, tag="tp", bufs=2)
                                for j in range(jlo, jhi + 1):
                                    nc.tensor.matmul(
                                        tpp[:, (j - jlo) * 128:(j - jlo + 1) * 128],
                                        lhsT=prow[:, j * 128:(j + 1) * 128],
                                        rhs=diag[:], start=True, stop=True)
                                pt_dst = pTq[q][:].rearrange("p (a b) -> p a b", a=4)[
                                    :, 0:nq, it * 128:(it + 1) * 128]
                                pt_src = tpp[:, :nq * 128].rearrange(
                                    "p (a b) -> p a b", b=128)
                                if (it + q) % 2:
                                    nc.scalar.copy(pt_dst, pt_src)
                                else:
                                    nc.vector.tensor_copy(pt_dst, pt_src)
                        yp = p2ps.tile([128, W], FP32, tag="yp", bufs=1)
                        jmax = ST
                        for j in range(jmax):
                            lo = 0
                            nc.tensor.matmul(
                                yp[:, lo:W],
                                lhsT=v_t[j][:, kv * H:(kv + 1) * H],
                                rhs=pTq[j // 4][:, (j % 4) * W + lo: (j % 4) * W + W],
                                start=(j == 0), stop=(j == jmax - 1))
                        nc.scalar.copy(yT_sb[h][:], yp[:])

                    for it in range(nt):
                        i = t0 + it
                        cg = next(ci for ci, (cs, cn) in enumerate(CC_CHUNKS)
                                  if cs <= i < cs + cn)
                        c_start, c_n = CC_CHUNKS[cg]
                        for dc in range(8):
                            op = p2ps.tile([128, 512], FP32, tag="op", bufs=1)
                            for hh in range(HEADS):
                                nc.tensor.matmul(
                                    op[:],
                                    lhsT=yT_sb[hh][:, it * 128:(it + 1) * 128],
                                    rhs=wo_sb[hh][:, dc * 512:(dc + 1) * 512],
                                    start=(hh == 0), stop=(hh == HEADS - 1))
                            oev = p2.tile([128, 512], BF, tag="oev", bufs=16)
                            nc.scalar.copy(oev[:], op[:])
                            nc.sync.dma_start(
                                cc_in[cg][(i - c_start) * 128:(i - c_start + 1) * 128,
                                          dc * 512:(dc + 1) * 512],
                                oev[:])
                        if i == c_start + c_n - 1:
                            nc.gpsimd.collective_compute(
                                "ReduceScatter", mybir.AluOpType.add,
                                replica_groups=[[0, 1, 2, 3], [4, 5, 6, 7]],
                                ins=[cc_in[cg].opt()], outs=[cc_out[cg].opt()])
                            orow = sum(cn * 32 for cs, cn in CC_CHUNKS[:cg])
                            nc.sync.dma_start(
                                out_sh.ap()[orow: orow + c_n * 32, :], cc_out[cg][:])

    nc.compile()
    return nc


_CANON_MASK = None


def _is_causal(mask: np.ndarray) -> bool:
    global _CANON_MASK
    if _CANON_MASK is None:
        _CANON_MASK = np.triu(np.full((S, S), -1e9, dtype=np.float32), k=1)
    return mask.shape == (S, S) and np.array_equal(mask, _CANON_MASK)


def _prepare(x, wq, wk, wv, wo, mask, sin, cos):
    causal = _is_causal(np.asarray(mask, dtype=np.float32))
    if causal not in _CACHE:
        _CACHE[causal] = _build_causal() if causal else _build_generic()
    nc = _CACHE[causal]
    chunks = CH_NEW if causal else CC_CHUNKS

    x = np.asarray(x, dtype=np.float32)
    scale = np.float32(H ** -0.5)
    cosT = np.ascontiguousarray(np.asarray(cos, np.float32).T)          # [H, S]
    sinT = np.asarray(sin, np.float32).T.copy()                          # [H, S]
    sinT[0:H // 2] = -sinT[0:H // 2]                                     # signed
    # per-core weight shards; head order = r-major over local kv heads
    in_maps = []
    for c in range(N_CORES):
        b, tp = c // TP, c % TP
        ks = slice(tp * KLOC, (tp + 1) * KLOC)
        wq_c = np.asarray(wq, np.float32)[:, :, ks, :].reshape(D, HEADS * H)
        wk_c = (np.asarray(wk, np.float32)[:, ks, :] * scale).reshape(D, KLOC * H)
        wv_c = np.asarray(wv, np.float32)[:, ks, :].reshape(D, KLOC * H)
        m = {
            "xP": x[b].reshape(2, S // 2, DT, 128).transpose(3, 0, 2, 1)
                     .reshape(128, 2 * DT * (S // 2)).astype(BF16),
            "wq": wq_c.reshape(DT, 128, HEADS, H).transpose(2, 1, 0, 3)
                      .reshape(HEADS * 128, DT * 128).astype(BF16),
            "wk": wk_c.reshape(DT, 128, KLOC, H).transpose(2, 1, 0, 3)
                      .reshape(KLOC * 128, DT * 128).astype(BF16),
            "wv": wv_c.reshape(DT, 128, KLOC * H).transpose(1, 0, 2)
                      .reshape(128, DT * KLOC * H).astype(BF16),
            "wo": np.asarray(wo, np.float32)[:, ks, :, :].reshape(HEADS * H, D).astype(BF16),
            "cosT": cosT,
            "sinST": sinT,
        }
        if causal:
            mdT = np.empty((128, S), np.float32)
            for i in range(ST):
                blk = mask[i * 128:(i + 1) * 128, i * 128:(i + 1) * 128]
                mdT[:, i * 128:(i + 1) * 128] = np.ascontiguousarray(blk.T)
            m["maskdT"] = mdT
        else:
            m["maskf"] = np.asarray(mask, np.float32)
        in_maps.append(m)
    return nc, in_maps, chunks


def _assemble(results, chunks):
    out = np.empty((B, S, D), dtype=np.float32)
    for c in range(N_CORES):
        b, tp = c // TP, c % TP
        sh = results[c]["out_shard"].astype(np.float32)
        orow = 0
        for cs, cn in chunks:
            rows = cn * 32
            out[b, cs * 128 + tp * rows: cs * 128 + (tp + 1) * rows, :] = \
                sh[orow: orow + rows]
            orow += rows
    return out


def kernel(x, wq, wk, wv, wo, mask, sin, cos):
    nc, in_maps, chunks = _prepare(x, wq, wk, wv, wo, mask, sin, cos)
    try:
        res = bass_utils.run_bass_kernel_spmd(nc, in_maps,
                                              core_ids=list(range(N_CORES)))
    except Exception:
        # transient device-side failures (e.g. NRT exec-unit errors) have
        # been observed once; a clean re-run succeeds.
        import time as _time
        _time.sleep(2.0)
        res = bass_utils.run_bass_kernel_spmd(nc, in_maps,
                                              core_ids=list(range(N_CORES)))
    return _assemble(res.results, chunks)


def _traced_run(x, wq, wk, wv, wo, mask, sin, cos):
    """Like kernel() but with NTFF tracing; returns BassKernelResults."""
    nc, in_maps, chunks = _prepare(x, wq, wk, wv, wo, mask, sin, cos)
    res = bass_utils.run_bass_kernel_spmd(nc, in_maps, core_ids=list(range(N_CORES)),
                                          trace=True)
    res.full_output = _assemble(res.results, chunks)
    return res
